# revision 42
# baseline (speedup 1.0000x reference)
"""Multi-head causal attention (B=2, S=2048, D=1024, H=16, hd=64) on 8 trn2 cores.

Sharding: core c handles batch b = c//4 and head-group g = c%4 (heads 4g..4g+4,
d-slice 256g..256g+256 of the QKV projections / Wo rows).  Each core computes a
partial out-projection [2048, 1024] in bf16; the host sums the 4 head-group
partials per batch in f32 and adds the bias.

v2 schedule: projections are interleaved WITH attention (j-streamed) so the
scalar engine (exp) and PE overlap instead of running in sequential phases.
 - inputs land in ~10 large DMAs (DMA issue on the sync engine costs ~600ns
   per instruction, so many small DMAs serialize badly)
 - attention groups run in increasing-g order, alternating pairs:
   (p0,g0),(p1,g0),(p0,g1),... so the first group only needs the j=0 column
   block of x
 - projection/out-projection units are drip-fed between attention kb-steps as
   PE filler while ACT (exp) paces the attention inner loop
 - softmax rowsum rides in v column 0 (ones), landing at PSUM partition 0 so
   reciprocal_approx_fast can read it directly; the reciprocal is broadcast
   across 64 partitions with a tiny k=1 f32r matmul instead of a DRAM bounce
 - partial out written bf16 (halves the output DMA)
"""

import sys
from collections import deque
from functools import partial

import numpy as np

for _p in ("/opt/trn_rl_repo",):
    if _p not in sys.path:
        sys.path.insert(0, _p)

import ml_dtypes

import concourse.bass as bass
import concourse.mybir as mybir
import concourse.tile as tile
from concourse import bacc
from concourse.bass_utils import run_bass_kernel_spmd
from concourse.masks import make_upper_triangular

BF16 = mybir.dt.bfloat16
F32 = mybir.dt.float32
F32R = mybir.dt.float32r

B, S, D, H, HD = 2, 2048, 1024, 16, 64
NCORES = 8
HPC = 4          # heads per core
DHC = HPC * HD   # 256: d-slice per core
P = 128
SB = S // P      # 16 seq blocks
KC = D // P      # 8 contraction chunks for projections
QG = 512         # q column group width
NQG = S // QG    # 4
VW = HD + 2      # 66: v cols per head (64 data + ones col for rowsum + 0 pad
                 # so M is even for the PE)

FILL_RATE = 0.45  # filler units per attention kb-step


def _build_body(ctx, tc, io):
    nc = tc.nc
    xT, wq, wk, wv, wo, out = (
        io["xT"], io["wq"], io["wk"], io["wv"], io["wo"], io["out"],
    )

    consts = ctx.enter_context(tc.tile_pool(name="consts", bufs=1))
    persist = ctx.enter_context(tc.tile_pool(name="persist", bufs=1))
    spool = ctx.enter_context(tc.tile_pool(name="spsum", bufs=2, space="PSUM"))
    cxpool = ctx.enter_context(tc.tile_pool(name="cxpsum", bufs=2, space="PSUM"))
    pjpool = ctx.enter_context(tc.tile_pool(name="pjpsum", bufs=2, space="PSUM"))
    espool = ctx.enter_context(tc.tile_pool(name="es", bufs=8))
    nrmpool = ctx.enter_context(tc.tile_pool(name="nrm", bufs=4))
    outpool = ctx.enter_context(tc.tile_pool(name="outsb", bufs=2))

    # triangular keep-mask for diagonal blocks: tri[i, j] = 1.0 iff j >= i
    tri = consts.tile([P, P], BF16, tag="tri", name="tri")
    make_upper_triangular(nc, tri[:], val=1.0, diag=True)
    ones64f = consts.tile([1, HD], F32, tag="ones64f", name="ones64f")
    nc.vector.memset(ones64f[:], 1.0)
    ones64 = consts.tile([1, HD], F32R, tag="ones64", name="ones64")
    nc.vector.tensor_copy(ones64[:], ones64f[:])

    # ---- inputs: one DMA per weight tensor, one per x column group ----
    wq_sb = persist.tile([P, KC, DHC], BF16, tag="wq", name="wq_sb")
    wk_sb = persist.tile([P, KC, DHC], BF16, tag="wk", name="wk_sb")
    wv_sb = persist.tile([P, KC, DHC], BF16, tag="wv", name="wv_sb")
    wo_sb = persist.tile([P, 2, D], BF16, tag="wo", name="wo_sb")
    def load_w(eng, dst, src, nch, w):
        src_ap = bass.AP(src.tensor, src.offset,
                         [[w, P], [P * w, nch], [1, w]])
        eng.dma_start(out=dst[:], in_=src_ap)

    load_w(nc.sync, wq_sb, wq, KC, DHC)

    xt = [persist.tile([P, KC, QG], BF16, tag=f"xt{j}", name=f"xt{j}")
          for j in range(NQG)]
    # j=0 lands in two half-chunks so the first projection's k-loop can start
    # as soon as chunks 0-3 arrive.  inputs are spread across the engines'
    # DMA queues (each engine issues on its own queue) so transfers overlap
    # instead of draining through one queue.
    for (k0, k1), eng in (((0, KC // 2), nc.sync), ((KC // 2, KC), nc.scalar)):
        src_ap = bass.AP(xT.tensor, xT.offset + k0 * P * S,
                         [[S, P], [P * S, k1 - k0], [1, QG]])
        eng.dma_start(out=xt[0][:, k0:k1, :], in_=src_ap)
    load_w(nc.sync, wv_sb, wv, KC, DHC)
    load_w(nc.scalar, wk_sb, wk, KC, DHC)
    for j, eng in ((1, nc.sync), (2, nc.scalar), (3, nc.sync)):
        src_ap = bass.AP(xT.tensor, xT.offset + j * QG,
                         [[S, P], [P * S, KC], [1, QG]])
        eng.dma_start(out=xt[j][:], in_=src_ap)
    wo_ap = bass.AP(wo.tensor, wo.offset, [[D, P], [P * D, 2], [1, D]])
    nc.scalar.dma_start(out=wo_sb[:], in_=wo_ap)

    # persistent tensors
    v_sb = [persist.tile([P, HPC, VW], BF16, tag=f"v{s}", name=f"v{s}")
            for s in range(SB)]
    qt = [persist.tile([P, S], BF16, tag=f"qt{i}", name=f"qt{i}") for i in range(2)]
    kt = [persist.tile([P, S], BF16, tag=f"kt{i}", name=f"kt{i}") for i in range(2)]
    ctxT = [persist.tile([P, S], BF16, tag=f"ctxT{i}", name=f"ctxT{i}")
            for i in range(2)]

    # ---- emission units ----
    def unit_qk(pair, t, j):
        # q (t=0) or k (t=1) projection: d-chunk `pair`, x column group j
        w_sb = wq_sb if t == 0 else wk_sb
        dst = (qt if t == 0 else kt)[pair]
        ps = pjpool.tile([P, QG], F32, tag="pj", name="pj")
        for k in range(KC):
            nc.tensor.matmul(
                ps[:],
                lhsT=w_sb[:, k, pair * P:(pair + 1) * P],
                rhs=xt[j][:, k, :],
                start=(k == 0),
                stop=(k == KC - 1),
            )
        nc.vector.tensor_copy(dst[:, j * QG:(j + 1) * QG], ps[:])

    def unit_v(sv, pool, tag):
        # seq blocks (2*sv, 2*sv+1) -> v natural layout [kpos, h, hd+2]
        ps = pool.tile([P, 2, DHC], F32, tag=tag, name="vp")
        for par in range(2):
            s = 2 * sv + par
            j, loc = divmod(s, NQG)
            for k in range(KC):
                nc.tensor.matmul(
                    ps[:, par, :],
                    lhsT=xt[j][:, k, loc * P:(loc + 1) * P],
                    rhs=wv_sb[:, k, :],
                    start=(k == 0),
                    stop=(k == KC - 1),
                )
        for par in range(2):
            s = 2 * sv + par
            src_ap = ps[:, par, :].rearrange("p (h d) -> p h d", h=HPC)
            nc.vector.tensor_copy(v_sb[s][:, :, 0:HD], src_ap)
            nc.vector.memset(v_sb[s][:, :, HD:VW], 1.0)
            nc.vector.memset(v_sb[s][:, :, HD + 1:VW], 0.0)

    def unit_outproj(m, pool, tag, tail):
        # partial out rows m*128..(m+1)*128.  mid-kernel: one DMA per block
        # on the sync queue (scalar is busy with exp).  tail: the drain is
        # device-HBM-bound (all 8 cores flush at once), so issue per
        # half-block right after each copy lands, alternating both queues.
        ot = outpool.tile([P, D], BF16, tag="ot", name="ot")
        for n2 in range(2):
            ps = pool.tile([P, QG], F32, tag=tag, name="op")
            for kc2 in range(2):
                nc.tensor.matmul(
                    ps[:],
                    lhsT=ctxT[kc2][:, m * P:(m + 1) * P],
                    rhs=wo_sb[:, kc2, n2 * QG:(n2 + 1) * QG],
                    start=(kc2 == 0),
                    stop=(kc2 == 1),
                )
            if tail and n2 == 0:
                nc.scalar.copy(ot[:, 0:QG], ps[:])
            else:
                nc.vector.tensor_copy(ot[:, n2 * QG:(n2 + 1) * QG], ps[:])
            if tail:
                eng = nc.sync if (2 * m + n2) % 2 == 0 else nc.scalar
                eng.dma_start(
                    out=out[m * P:(m + 1) * P, n2 * QG:(n2 + 1) * QG],
                    in_=ot[:, n2 * QG:(n2 + 1) * QG])
        if not tail:
            nc.sync.dma_start(out=out[m * P:(m + 1) * P, :], in_=ot[:])

    # Schraudolph exp for the vector engine: bf16 bits of exp(s/8) are
    # approximately round(A*s + B); writing the int16 bit pattern into the
    # bf16 es tile gives a ~±4% per-weight approximation whose error largely
    # cancels between the softmax numerator and denominator.  Used only for
    # odd full blocks of the g=3 groups to take them off the ACT critical
    # path (ACT otherwise paces the final ~35us of the kernel).
    EXP_A = 128.0 * 1.4426950408889634 / 8.0
    EXP_B = 16256.0 - 7.63

    def attention_group(pair, g, last=False):
        # generator: one yield per kb step so the scheduler can add filler
        nkb = 4 * g + 4
        cxs = [cxpool.tile([VW, QG], F32, tag="cx", name="cx") for _ in range(2)]
        pend = deque()
        for kb in range(nkb + 1):
            if kb < nkb:
                c0 = P * (kb - 4 * g) if kb >= 4 * g else 0
                sp_t = spool.tile([P, 2, QG], F32, tag="sp", name="sp")
                for hh in range(2):
                    nc.tensor.matmul(
                        sp_t[:, hh, c0:QG],
                        lhsT=kt[pair][hh * HD:(hh + 1) * HD, kb * P:(kb + 1) * P],
                        rhs=qt[pair][hh * HD:(hh + 1) * HD,
                                     g * QG + c0:(g + 1) * QG],
                        start=True,
                        stop=True,
                    )
                es_t = espool.tile([P, 2, QG], BF16, tag="es", name="es")
                if g == 3 and kb < 4 * g and kb % 2 == 1:
                    nc.vector.tensor_scalar(
                        es_t[:, :, :].bitcast(mybir.dt.int16), sp_t[:, :, :],
                        EXP_A, EXP_B,
                        mybir.AluOpType.mult, mybir.AluOpType.add,
                    )
                else:
                    nc.scalar.activation(
                        es_t[:, :, c0:QG], sp_t[:, :, c0:QG],
                        mybir.ActivationFunctionType.Exp, scale=0.125,
                    )
                if kb >= 4 * g:
                    dst = es_t[:, :, c0:c0 + P]
                    t_ap = tri[:]
                    tri_b = bass.AP(t_ap.tensor, t_ap.offset,
                                    [t_ap.ap[0], [0, 2], t_ap.ap[1]])
                    nc.vector.tensor_mul(dst, dst, tri_b)
                pend.append((kb, es_t, c0))
            if kb >= 1:
                pkb, pes, pc0 = pend.popleft()
                for hh in range(2):
                    h = 2 * pair + hh
                    nc.tensor.matmul(
                        cxs[hh][:, pc0:QG],
                        lhsT=v_sb[pkb][:, h, :],
                        rhs=pes[:, hh, pc0:QG],
                        start=(pkb == 0),
                        stop=(pkb == nkb - 1),
                    )
            if kb < nkb:
                yield
        for hh in range(2):
            # rowsum sits at psum partition 64: stage to sbuf partition 0
            # (f32r-rounding copy), broadcast the raw rowsum across the 64
            # ctx partitions with a k=1 f32r matmul, approx-reciprocal on the
            # broadcast (free-dim bound: same cost as on one row), multiply
            rs = nrmpool.tile([1, QG], F32R, tag="rs", name="rs")
            if last:
                # keep the tail-critical DVE chain short: stage on scalar
                nc.scalar.copy(rs[:], cxs[hh][HD:HD + 1, :])
            else:
                nc.vector.tensor_copy(rs[:], cxs[hh][HD:HD + 1, :])
            bc = pjpool.tile([HD, QG], F32, tag="pj", name="bc")
            nc.tensor.matmul(bc[:], lhsT=ones64[:], rhs=rs[:],
                             start=True, stop=True)
            rb = nrmpool.tile([HD, QG], F32, tag="rb", name="rb")
            nc.vector.reciprocal_approx_fast(rb[:], bc[:])
            nc.vector.tensor_mul(
                ctxT[pair][hh * HD:(hh + 1) * HD, g * QG:(g + 1) * QG],
                cxs[hh][0:HD, :],
                rb[:],
            )

    # ---- emission schedule ----
    # minimal pre-attention: just what group (0,0) needs.  everything else is
    # drip-fed between attention kb-steps: projection units at the rate their
    # dependency deadlines demand, out-projections at a slow steady rate so
    # the 4MB of output DMA spreads across the kernel instead of piling up
    # at the tail.
    unit_qk(0, 0, 0)
    unit_v(0, cxpool, "cx")
    unit_qk(0, 1, 0)
    unit_v(1, cxpool, "cx")

    filler = deque()
    filler.append(partial(unit_qk, 1, 0, 0))
    filler.append(partial(unit_qk, 1, 1, 0))
    for j in (1, 2, 3):
        filler.append(partial(unit_qk, 0, 0, j))
        filler.append(partial(unit_v, 2 * j, pjpool, "pj"))
        filler.append(partial(unit_qk, 0, 1, j))
        filler.append(partial(unit_v, 2 * j + 1, pjpool, "pj"))
        filler.append(partial(unit_qk, 1, 0, j))
        filler.append(partial(unit_qk, 1, 1, j))
    groups = [(0, 0), (1, 0), (0, 1), (1, 1), (0, 2), (1, 2), (0, 3), (1, 3)]
    # (deadline step, filler prefix that must be drained by then)
    deadlines = [(4, 2), (8, 6), (16, 8), (24, 12), (36, 14),
                 (48, 18), (64, 20)]
    req = {groups[i + 1]: deadlines[i][1] for i in range(7)}
    drained = 0
    step = 0
    outq = deque()
    acc_p = acc_o = 0.0
    for (p, g) in groups:
        while drained < req.get((p, g), 0):
            filler.popleft()()
            drained += 1
        for _ in attention_group(p, g, last=(p, g) == groups[-1]):
            step += 1
            need = max((n - drained) / max(sd - step, 1)
                       for sd, n in deadlines + [(80, 20)])
            acc_p += max(need, 0.0)
            while acc_p >= 1.0 and filler:
                acc_p -= 1.0
                filler.popleft()()
                drained += 1
            acc_o += 0.25 + (0.2 if not filler else 0.0)
            while acc_o >= 1.0 and outq:
                acc_o -= 1.0
                outq.popleft()()
        if p == 1 and g < 3:
            for m in range(4 * g, 4 * g + 4):
                outq.append(partial(unit_outproj, m, pjpool, "pj", False))
    while filler:
        filler.popleft()()
    while outq:
        outq.popleft()()
    for m in range(12, 16):
        unit_outproj(m, spool, "sp", True)


def build_nc():
    from contextlib import ExitStack

    nc = bacc.Bacc()
    io = {
        "xT": nc.dram_tensor("xT", [D, S], BF16, kind="ExternalInput").ap(),
        "wq": nc.dram_tensor("wq", [D, DHC], BF16, kind="ExternalInput").ap(),
        "wk": nc.dram_tensor("wk", [D, DHC], BF16, kind="ExternalInput").ap(),
        "wv": nc.dram_tensor("wv", [D, DHC], BF16, kind="ExternalInput").ap(),
        "wo": nc.dram_tensor("wo", [DHC, D], BF16, kind="ExternalInput").ap(),
        "out": nc.dram_tensor("out", [S, D], BF16, kind="ExternalOutput").ap(),
    }
    with tile.TileContext(nc) as tc:
        with ExitStack() as ctx:
            _build_body(ctx, tc, io)
    nc.finalize()
    return nc


_NC = None


def _get_nc():
    global _NC
    if _NC is None:
        _NC = build_nc()
    return _NC


def make_in_maps(x, Wq, Wk, Wv, Wo):
    bf = ml_dtypes.bfloat16
    x = np.asarray(x, dtype=np.float32)
    in_maps = []
    xTs = [np.ascontiguousarray(x[b].T).astype(bf) for b in range(B)]
    for c in range(NCORES):
        b, g = divmod(c, 4)
        sl = slice(DHC * g, DHC * (g + 1))
        in_maps.append({
            "xT": xTs[b],
            "wq": np.ascontiguousarray(np.asarray(Wq, np.float32)[:, sl]).astype(bf),
            "wk": np.ascontiguousarray(np.asarray(Wk, np.float32)[:, sl]).astype(bf),
            "wv": np.ascontiguousarray(np.asarray(Wv, np.float32)[:, sl]).astype(bf),
            "wo": np.ascontiguousarray(np.asarray(Wo, np.float32)[sl, :]).astype(bf),
        })
    return in_maps


def run(in_maps, trace=False, **kw):
    return run_bass_kernel_spmd(_get_nc(), in_maps, list(range(NCORES)),
                                trace=trace, **kw)


def kernel(x, Wq, Wk, Wv, Wo, bo):
    res = run(make_in_maps(x, Wq, Wk, Wv, Wo)).results
    bo = np.asarray(bo, np.float32)
    out = np.empty((B, S, D), np.float32)
    for b in range(B):
        acc = res[4 * b]["out"].astype(np.float32)
        for g in range(1, 4):
            acc = acc + res[4 * b + g]["out"].astype(np.float32)
        out[b] = acc + bo[None, :]
    return out


# revision 44
# speedup vs baseline: 1.0655x; 1.0655x over previous
"""Multi-head causal attention (B=2, S=2048, D=1024, H=16, hd=64) on 8 trn2 cores.

Sharding: core c handles batch b = c//4 and head-group g = c%4 (heads 4g..4g+4,
d-slice 256g..256g+256 of the QKV projections / Wo rows).  Each core computes a
partial out-projection [2048, 1024] in bf16; the host sums the 4 head-group
partials per batch in f32 and adds the bias.

v2 schedule: projections are interleaved WITH attention (j-streamed) so the
scalar engine (exp) and PE overlap instead of running in sequential phases.
 - inputs land in ~10 large DMAs (DMA issue on the sync engine costs ~600ns
   per instruction, so many small DMAs serialize badly)
 - attention groups run in increasing-g order, alternating pairs:
   (p0,g0),(p1,g0),(p0,g1),... so the first group only needs the j=0 column
   block of x
 - projection/out-projection units are drip-fed between attention kb-steps as
   PE filler while ACT (exp) paces the attention inner loop
 - softmax rowsum rides in v column 0 (ones), landing at PSUM partition 0 so
   reciprocal_approx_fast can read it directly; the reciprocal is broadcast
   across 64 partitions with a tiny k=1 f32r matmul instead of a DRAM bounce
 - partial out written bf16 (halves the output DMA)
"""

import sys
from collections import deque
from functools import partial

import numpy as np

for _p in ("/opt/trn_rl_repo",):
    if _p not in sys.path:
        sys.path.insert(0, _p)

import ml_dtypes

import concourse.bass as bass
import concourse.mybir as mybir
import concourse.tile as tile
from concourse import bacc
from concourse.bass_utils import run_bass_kernel_spmd
from concourse.masks import make_upper_triangular

BF16 = mybir.dt.bfloat16
F32 = mybir.dt.float32
F32R = mybir.dt.float32r

B, S, D, H, HD = 2, 2048, 1024, 16, 64
NCORES = 8
HPC = 4          # heads per core
DHC = HPC * HD   # 256: d-slice per core
P = 128
SB = S // P      # 16 seq blocks
KC = D // P      # 8 contraction chunks for projections
QG = 512         # q column group width
NQG = S // QG    # 4
VW = HD + 2      # 66: v cols per head (64 data + ones col for rowsum + 0 pad
                 # so M is even for the PE)

FILL_RATE = 0.45  # filler units per attention kb-step


def _build_body(ctx, tc, io):
    nc = tc.nc
    xT, wq, wk, wv, wo, out = (
        io["xT"], io["wq"], io["wk"], io["wv"], io["wo"], io["out"],
    )

    consts = ctx.enter_context(tc.tile_pool(name="consts", bufs=1))
    persist = ctx.enter_context(tc.tile_pool(name="persist", bufs=1))
    spool = ctx.enter_context(tc.tile_pool(name="spsum", bufs=2, space="PSUM"))
    cxpool = ctx.enter_context(tc.tile_pool(name="cxpsum", bufs=2, space="PSUM"))
    pjpool = ctx.enter_context(tc.tile_pool(name="pjpsum", bufs=2, space="PSUM"))
    espool = ctx.enter_context(tc.tile_pool(name="es", bufs=8))
    nrmpool = ctx.enter_context(tc.tile_pool(name="nrm", bufs=4))
    outpool = ctx.enter_context(tc.tile_pool(name="outsb", bufs=2))

    # triangular keep-mask for diagonal blocks: tri[i, j] = 1.0 iff j >= i
    tri = consts.tile([P, P], BF16, tag="tri", name="tri")
    make_upper_triangular(nc, tri[:], val=1.0, diag=True)
    ones64f = consts.tile([1, HD], F32, tag="ones64f", name="ones64f")
    nc.vector.memset(ones64f[:], 1.0)
    ones64 = consts.tile([1, HD], F32R, tag="ones64", name="ones64")
    nc.vector.tensor_copy(ones64[:], ones64f[:])

    # ---- inputs: one DMA per weight tensor, one per x column group ----
    wq_sb = persist.tile([P, KC, DHC], BF16, tag="wq", name="wq_sb")
    wk_sb = persist.tile([P, KC, DHC], BF16, tag="wk", name="wk_sb")
    wv_sb = persist.tile([P, KC, DHC], BF16, tag="wv", name="wv_sb")
    wo_sb = persist.tile([P, 2, D], BF16, tag="wo", name="wo_sb")
    def load_w(eng, dst, src, nch, w):
        src_ap = bass.AP(src.tensor, src.offset,
                         [[w, P], [P * w, nch], [1, w]])
        eng.dma_start(out=dst[:], in_=src_ap)

    load_w(nc.sync, wq_sb, wq, KC, DHC)

    xt = [persist.tile([P, KC, QG], BF16, tag=f"xt{j}", name=f"xt{j}")
          for j in range(NQG)]
    # j=0 lands in two half-chunks so the first projection's k-loop can start
    # as soon as chunks 0-3 arrive.  inputs are spread across the engines'
    # DMA queues (each engine issues on its own queue) so transfers overlap
    # instead of draining through one queue.
    for (k0, k1), eng in (((0, KC // 2), nc.sync), ((KC // 2, KC), nc.scalar)):
        src_ap = bass.AP(xT.tensor, xT.offset + k0 * P * S,
                         [[S, P], [P * S, k1 - k0], [1, QG]])
        eng.dma_start(out=xt[0][:, k0:k1, :], in_=src_ap)
    load_w(nc.scalar, wv_sb, wv, KC, DHC)
    load_w(nc.scalar, wk_sb, wk, KC, DHC)
    for j, eng in ((1, nc.sync), (2, nc.scalar), (3, nc.sync)):
        src_ap = bass.AP(xT.tensor, xT.offset + j * QG,
                         [[S, P], [P * S, KC], [1, QG]])
        eng.dma_start(out=xt[j][:], in_=src_ap)
    wo_ap = bass.AP(wo.tensor, wo.offset, [[D, P], [P * D, 2], [1, D]])
    nc.scalar.dma_start(out=wo_sb[:], in_=wo_ap)

    # persistent tensors
    v_sb = [persist.tile([P, HPC, VW], BF16, tag=f"v{s}", name=f"v{s}")
            for s in range(SB)]
    qt = [persist.tile([P, S], BF16, tag=f"qt{i}", name=f"qt{i}") for i in range(2)]
    kt = [persist.tile([P, S], BF16, tag=f"kt{i}", name=f"kt{i}") for i in range(2)]
    ctxT = [persist.tile([P, S], BF16, tag=f"ctxT{i}", name=f"ctxT{i}")
            for i in range(2)]

    # ---- emission units ----
    def unit_qk(pair, t, j):
        # q (t=0) or k (t=1) projection: d-chunk `pair`, x column group j
        w_sb = wq_sb if t == 0 else wk_sb
        dst = (qt if t == 0 else kt)[pair]
        ps = pjpool.tile([P, QG], F32, tag="pj", name="pj")
        for k in range(KC):
            nc.tensor.matmul(
                ps[:],
                lhsT=w_sb[:, k, pair * P:(pair + 1) * P],
                rhs=xt[j][:, k, :],
                start=(k == 0),
                stop=(k == KC - 1),
            )
        nc.vector.tensor_copy(dst[:, j * QG:(j + 1) * QG], ps[:])

    def unit_v(sv, pool, tag):
        # seq blocks (2*sv, 2*sv+1) -> v natural layout [kpos, h, hd+2]
        ps = pool.tile([P, 2, DHC], F32, tag=tag, name="vp")
        for par in range(2):
            s = 2 * sv + par
            j, loc = divmod(s, NQG)
            for k in range(KC):
                nc.tensor.matmul(
                    ps[:, par, :],
                    lhsT=xt[j][:, k, loc * P:(loc + 1) * P],
                    rhs=wv_sb[:, k, :],
                    start=(k == 0),
                    stop=(k == KC - 1),
                )
        for par in range(2):
            s = 2 * sv + par
            src_ap = ps[:, par, :].rearrange("p (h d) -> p h d", h=HPC)
            nc.vector.tensor_copy(v_sb[s][:, :, 0:HD], src_ap)
            nc.vector.memset(v_sb[s][:, :, HD:VW], 1.0)
            nc.vector.memset(v_sb[s][:, :, HD + 1:VW], 0.0)

    def unit_outproj(m, pool, tag, tail):
        # partial out rows m*128..(m+1)*128.  mid-kernel: one DMA per block
        # on the sync queue (scalar is busy with exp).  tail: the drain is
        # device-HBM-bound (all 8 cores flush at once), so issue per
        # half-block right after each copy lands, alternating both queues.
        ot = outpool.tile([P, D], BF16, tag="ot", name="ot")
        for n2 in range(2):
            ps = pool.tile([P, QG], F32, tag=tag, name="op")
            for kc2 in range(2):
                nc.tensor.matmul(
                    ps[:],
                    lhsT=ctxT[kc2][:, m * P:(m + 1) * P],
                    rhs=wo_sb[:, kc2, n2 * QG:(n2 + 1) * QG],
                    start=(kc2 == 0),
                    stop=(kc2 == 1),
                )
            if tail and n2 == 0:
                nc.scalar.copy(ot[:, 0:QG], ps[:])
            else:
                nc.vector.tensor_copy(ot[:, n2 * QG:(n2 + 1) * QG], ps[:])
            if tail:
                eng = nc.sync if (2 * m + n2) % 2 == 0 else nc.scalar
                eng.dma_start(
                    out=out[m * P:(m + 1) * P, n2 * QG:(n2 + 1) * QG],
                    in_=ot[:, n2 * QG:(n2 + 1) * QG])
        if not tail:
            nc.sync.dma_start(out=out[m * P:(m + 1) * P, :], in_=ot[:])

    def attention_group(pair, g, last=False):
        # generator: one yield per kb step so the scheduler can add filler
        nkb = 4 * g + 4
        cxs = [cxpool.tile([VW, QG], F32, tag="cx", name="cx") for _ in range(2)]
        pend = deque()
        for kb in range(nkb + 1):
            if kb < nkb:
                c0 = P * (kb - 4 * g) if kb >= 4 * g else 0
                sp_t = spool.tile([P, 2, QG], F32, tag="sp", name="sp")
                for hh in range(2):
                    nc.tensor.matmul(
                        sp_t[:, hh, c0:QG],
                        lhsT=kt[pair][hh * HD:(hh + 1) * HD, kb * P:(kb + 1) * P],
                        rhs=qt[pair][hh * HD:(hh + 1) * HD,
                                     g * QG + c0:(g + 1) * QG],
                        start=True,
                        stop=True,
                    )
                es_t = espool.tile([P, 2, QG], BF16, tag="es", name="es")
                nc.scalar.activation(
                    es_t[:, :, c0:QG], sp_t[:, :, c0:QG],
                    mybir.ActivationFunctionType.Exp, scale=0.125,
                )
                if kb >= 4 * g:
                    dst = es_t[:, :, c0:c0 + P]
                    t_ap = tri[:]
                    tri_b = bass.AP(t_ap.tensor, t_ap.offset,
                                    [t_ap.ap[0], [0, 2], t_ap.ap[1]])
                    nc.vector.tensor_mul(dst, dst, tri_b)
                pend.append((kb, es_t, c0))
            if kb >= 1:
                pkb, pes, pc0 = pend.popleft()
                for hh in range(2):
                    h = 2 * pair + hh
                    nc.tensor.matmul(
                        cxs[hh][:, pc0:QG],
                        lhsT=v_sb[pkb][:, h, :],
                        rhs=pes[:, hh, pc0:QG],
                        start=(pkb == 0),
                        stop=(pkb == nkb - 1),
                    )
            if kb < nkb:
                yield
        for hh in range(2):
            # rowsum sits at psum partition 64: stage to sbuf partition 0
            # (f32r-rounding copy), broadcast the raw rowsum across the 64
            # ctx partitions with a k=1 f32r matmul, approx-reciprocal on the
            # broadcast (free-dim bound: same cost as on one row), multiply
            rs = nrmpool.tile([1, QG], F32R, tag="rs", name="rs")
            if last:
                # keep the tail-critical DVE chain short: stage on scalar
                nc.scalar.copy(rs[:], cxs[hh][HD:HD + 1, :])
            else:
                nc.vector.tensor_copy(rs[:], cxs[hh][HD:HD + 1, :])
            bc = pjpool.tile([HD, QG], F32, tag="pj", name="bc")
            nc.tensor.matmul(bc[:], lhsT=ones64[:], rhs=rs[:],
                             start=True, stop=True)
            rb = nrmpool.tile([HD, QG], F32, tag="rb", name="rb")
            nc.vector.reciprocal_approx_fast(rb[:], bc[:])
            nc.vector.tensor_mul(
                ctxT[pair][hh * HD:(hh + 1) * HD, g * QG:(g + 1) * QG],
                cxs[hh][0:HD, :],
                rb[:],
            )

    # ---- emission schedule ----
    # minimal pre-attention: just what group (0,0) needs.  everything else is
    # drip-fed between attention kb-steps: projection units at the rate their
    # dependency deadlines demand, out-projections at a slow steady rate so
    # the 4MB of output DMA spreads across the kernel instead of piling up
    # at the tail.
    unit_qk(0, 0, 0)
    unit_v(0, cxpool, "cx")
    unit_qk(0, 1, 0)
    unit_v(1, cxpool, "cx")

    filler = deque()
    filler.append(partial(unit_qk, 1, 0, 0))
    filler.append(partial(unit_qk, 1, 1, 0))
    for j in (1, 2, 3):
        filler.append(partial(unit_qk, 0, 0, j))
        filler.append(partial(unit_v, 2 * j, pjpool, "pj"))
        filler.append(partial(unit_qk, 0, 1, j))
        filler.append(partial(unit_v, 2 * j + 1, pjpool, "pj"))
        filler.append(partial(unit_qk, 1, 0, j))
        filler.append(partial(unit_qk, 1, 1, j))
    groups = [(0, 0), (1, 0), (0, 1), (1, 1), (0, 2), (1, 2), (0, 3), (1, 3)]
    # (deadline step, filler prefix that must be drained by then)
    deadlines = [(4, 2), (8, 6), (16, 8), (24, 12), (36, 14),
                 (48, 18), (64, 20)]
    req = {groups[i + 1]: deadlines[i][1] for i in range(7)}
    drained = 0
    step = 0
    outq = deque()
    acc_p = acc_o = 0.0
    for (p, g) in groups:
        while drained < req.get((p, g), 0):
            filler.popleft()()
            drained += 1
        for _ in attention_group(p, g, last=(p, g) == groups[-1]):
            step += 1
            need = max((n - drained) / max(sd - step, 1)
                       for sd, n in deadlines + [(80, 20)])
            acc_p += max(need, 0.0)
            while acc_p >= 1.0 and filler:
                acc_p -= 1.0
                filler.popleft()()
                drained += 1
            acc_o += 0.25 + (0.2 if not filler else 0.0)
            while acc_o >= 1.0 and outq:
                acc_o -= 1.0
                outq.popleft()()
        if p == 1 and g < 3:
            for m in range(4 * g, 4 * g + 4):
                outq.append(partial(unit_outproj, m, pjpool, "pj", False))
    while filler:
        filler.popleft()()
    while outq:
        outq.popleft()()
    for m in range(12, 16):
        unit_outproj(m, spool, "sp", True)


def build_nc():
    from contextlib import ExitStack

    nc = bacc.Bacc()
    io = {
        "xT": nc.dram_tensor("xT", [D, S], BF16, kind="ExternalInput").ap(),
        "wq": nc.dram_tensor("wq", [D, DHC], BF16, kind="ExternalInput").ap(),
        "wk": nc.dram_tensor("wk", [D, DHC], BF16, kind="ExternalInput").ap(),
        "wv": nc.dram_tensor("wv", [D, DHC], BF16, kind="ExternalInput").ap(),
        "wo": nc.dram_tensor("wo", [DHC, D], BF16, kind="ExternalInput").ap(),
        "out": nc.dram_tensor("out", [S, D], BF16, kind="ExternalOutput").ap(),
    }
    with tile.TileContext(nc) as tc:
        with ExitStack() as ctx:
            _build_body(ctx, tc, io)
    nc.finalize()
    return nc


_NC = None


def _get_nc():
    global _NC
    if _NC is None:
        _NC = build_nc()
    return _NC


def make_in_maps(x, Wq, Wk, Wv, Wo):
    bf = ml_dtypes.bfloat16
    x = np.asarray(x, dtype=np.float32)
    in_maps = []
    xTs = [np.ascontiguousarray(x[b].T).astype(bf) for b in range(B)]
    for c in range(NCORES):
        b, g = divmod(c, 4)
        sl = slice(DHC * g, DHC * (g + 1))
        in_maps.append({
            "xT": xTs[b],
            "wq": np.ascontiguousarray(np.asarray(Wq, np.float32)[:, sl]).astype(bf),
            "wk": np.ascontiguousarray(np.asarray(Wk, np.float32)[:, sl]).astype(bf),
            "wv": np.ascontiguousarray(np.asarray(Wv, np.float32)[:, sl]).astype(bf),
            "wo": np.ascontiguousarray(np.asarray(Wo, np.float32)[sl, :]).astype(bf),
        })
    return in_maps


def run(in_maps, trace=False, **kw):
    return run_bass_kernel_spmd(_get_nc(), in_maps, list(range(NCORES)),
                                trace=trace, **kw)


def kernel(x, Wq, Wk, Wv, Wo, bo):
    res = run(make_in_maps(x, Wq, Wk, Wv, Wo)).results
    bo = np.asarray(bo, np.float32)
    out = np.empty((B, S, D), np.float32)
    for b in range(B):
        acc = res[4 * b]["out"].astype(np.float32)
        for g in range(1, 4):
            acc = acc + res[4 * b + g]["out"].astype(np.float32)
        out[b] = acc + bo[None, :]
    return out
